# revision 12
# baseline (speedup 1.0000x reference)
"""AdaAT (adaptive affine transform) Trainium2 kernel.

Reference computation: tiny MLP head produces per-(batch,channel) rotation/
scale/translation; each channel of feature_map [4,256,64,64] is warped by a
2D affine grid_sample (trilinear in 3D, but the z-axis taps are static and
only mix adjacent channels, so z reduces to a fixed per-channel blend).

Device algorithm (no gather hardware worth using on TRN2 -- measured
ap_gather at ~600 cyc/idx):  exact bilinear sampling as PE matmuls.
For output pixel p of channel c:
    out[p] = sum_y sum_x tri(y - py[c,p]) * tri(x - px[c,p]) * B_c[y,x]
with tri(t) = relu(1 - |t|) and B_c the z-blended slice.  Zero padding is
automatic (taps outside [0,63] simply have no row/column).
Per channel-pair (2 channels share every matmul via block structure):
  1. K=2 "broadcast" matmul replicates py (resp. px) rows across 64
     partitions per channel -> PSUM [128, N]
  2. tri() built elementwise (ACT for y, DVE for x)
  3. K=128 block-diagonal matmul with the blended slices contracts y
  4. DVE multiply by the x-tri weights
  5. K=128 ones matmul (column sum) contracts x -> out rows [2, N]

Sharding: 8 cores = 4 batches x 2 channel-halves (z-taps of each half stay
inside the half, so shards are independent).
"""

import numpy as np

import concourse.bass as bass
import concourse.tile as tile
from concourse import bacc, mybir
from concourse.bass_utils import run_bass_kernel_spmd

F32 = mybir.dt.float32
AF = mybir.ActivationFunctionType
ALU = mybir.AluOpType

PI = 3.14159  # matches reference.py
B, C, H, W = 4, 256, 64, 64
NPIX = H * W            # 4096
HALF = 128              # channels per core
NPAIR = HALF // 2       # 64
CHUNK = 512
NCHUNK = NPIX // CHUNK  # 8


# ---------------------------------------------------------------- host consts
def _consts():
    c = {}
    pix = np.arange(NPIX)
    iota3 = np.stack([
        (pix % W).astype(np.float32),          # w
        (pix // W).astype(np.float32),         # h
        np.ones(NPIX, np.float32),             # 1
    ])                                          # [3, 4096]
    c["iota3"] = iota3

    bcy = np.zeros((2, 128), np.float32)
    bcy[0, :64] = 1.0
    bcy[1, 64:] = 1.0
    c["bcy"] = bcy                              # K=2 broadcast lhsT

    osel = np.zeros((128, 32, 64), np.float32)
    for v in range(32):
        osel[:64, v, 2 * v] = 1.0
        osel[64:, v, 2 * v + 1] = 1.0
    c["osel"] = osel.reshape(128, 2048)         # column-sum selector lhsT

    p = np.arange(128)
    c["ycol"] = (p % 64).astype(np.float32)[:, None]      # [128,1]
    c["negy"] = (-(p % 64)).astype(np.float32)[:, None]   # [128,1]
    c["ident"] = np.eye(128, dtype=np.float32)
    y3h = np.zeros((3, 64), np.float32)
    y3h[2, :] = -np.arange(64, dtype=np.float32)
    c["y3h"] = y3h          # constant rows (0, 0, -y) folded into lhsT
    return c


def _zcoef(half):
    """Per-channel z-blend coefficients, laid out [128 part, 64 pair, 64 x].

    partition p < 64 -> channel 2r of the pair, p >= 64 -> channel 2r+1.
    Returns (coef_cur, coef_other) as [128, 4096] f32 plus the channel
    indices of the "other" (prev for lower half / next for upper) slice.
    """
    j = np.arange(HALF)                       # local channel
    d = 128 * half + j                        # global channel
    if half == 0:
        cur = 0.5 + d / 255.0
        oth = 0.5 - d / 255.0
        oth[0] = 0.0                          # z tap -1 is masked
        oidx = np.clip(j - 1, 0, None)
    else:
        cur = 1.5 - d / 255.0
        oth = d / 255.0 - 0.5
        oth[-1] = 0.0                         # z tap 256 is masked
        oidx = np.clip(j + 1, None, HALF - 1)

    def layout(v):
        # v[channel] -> [128 p, 64 r, 64 x]: value of channel 2r + (p>=64)
        t = np.zeros((128, NPAIR, W), np.float32)
        r = np.arange(NPAIR)
        t[:64, :, :] = v[2 * r][None, :, None]
        t[64:, :, :] = v[2 * r + 1][None, :, None]
        return t.reshape(128, NPIX)

    return layout(cur), layout(oth), oidx


def _fm_layout(fm_half):
    """[128ch, 64, 64] -> [128 part, 64 pair, 64 x] with part=(half64, y)."""
    t = fm_half.reshape(NPAIR, 2, H, W)       # r, half, y, x
    return np.ascontiguousarray(
        t.transpose(1, 2, 0, 3).reshape(128, NPIX))


def shard_inputs(feature_map, para_code, W_c, b_c, W_s, b_s, W_r, b_r, W_t, b_t):
    consts = _consts()
    in_maps = []
    for core in range(8):
        b_i, half = core // 2, core % 2
        ch = slice(128 * half, 128 * (half + 1))
        fm = feature_map[b_i, ch]                       # [128, 64, 64]
        zcc, zco, oidx = _zcoef(half)
        fmo = feature_map[b_i, ch][oidx]                # neighbor slices
        cols = 2 * (128 * half + np.arange(HALF))
        m = dict(consts)
        m.update(
            fmt=_fm_layout(fm),
            fmo=_fm_layout(fmo),
            zcc=zcc,
            zco=zco,
            para=para_code[b_i].astype(np.float32)[:, None],     # [256,1]
            Wc=W_c.astype(np.float32),
            bc=b_c.astype(np.float32)[:, None],
            Ws=np.ascontiguousarray(W_s[:, ch]),
            bs=np.ascontiguousarray(b_s[ch])[:, None],
            Wr=np.ascontiguousarray(W_r[:, ch]),
            br=np.ascontiguousarray(b_r[ch])[:, None],
            Wtx=np.ascontiguousarray(W_t[:, cols]),
            btx=np.ascontiguousarray(b_t[cols])[:, None],
            Wty=np.ascontiguousarray(W_t[:, cols + 1]),
            bty=np.ascontiguousarray(b_t[cols + 1])[:, None],
        )
        in_maps.append({k: np.ascontiguousarray(v, dtype=np.float32)
                        for k, v in m.items()})
    return in_maps


# ---------------------------------------------------------------- device build
def build_nc():
    nc = bacc.Bacc("TRN2", target_bir_lowering=False, debug=False,
                   enable_asserts=False, num_devices=8)

    def din(name, shape):
        return nc.dram_tensor(name, shape, F32, kind="ExternalInput")

    fmt_d = din("fmt", [128, NPIX])
    fmo_d = din("fmo", [128, NPIX])
    zcc_d = din("zcc", [128, NPIX])
    zco_d = din("zco", [128, NPIX])
    para_d = din("para", [256, 1])
    Wc_d = din("Wc", [256, 256])
    bc_d = din("bc", [256, 1])
    Ws_d = din("Ws", [256, 128])
    bs_d = din("bs", [128, 1])
    Wr_d = din("Wr", [256, 128])
    br_d = din("br", [128, 1])
    Wtx_d = din("Wtx", [256, 128])
    btx_d = din("btx", [128, 1])
    Wty_d = din("Wty", [256, 128])
    bty_d = din("bty", [128, 1])
    iota3_d = din("iota3", [3, NPIX])
    bcy_d = din("bcy", [2, 128])
    osel_d = din("osel", [128, 2048])
    ycol_d = din("ycol", [128, 1])
    negy_d = din("negy", [128, 1])
    ident_d = din("ident", [128, 128])
    y3h_d = din("y3h", [3, 64])
    out_d = nc.dram_tensor("out", [128, NPIX], F32, kind="ExternalOutput")

    with tile.TileContext(nc) as tc:
        with (
            tc.tile_pool(name="const", bufs=1) as cpool,
            tc.tile_pool(name="mlp", bufs=1) as mpool,
            tc.tile_pool(name="big", bufs=1) as bpool,
            tc.tile_pool(name="work", bufs=3) as wpool,
        ):
            mlp_psum_scope = tc.tile_pool(name="mlpp", bufs=2, space="PSUM")
            mpsum = mlp_psum_scope.__enter__()
            # ---- load constants / weights
            def load(pool, dram, shape):
                t = pool.tile(shape, F32, tag=dram.name)
                nc.sync.dma_start(t[:], dram[:, :])
                return t

            iota3 = load(cpool, iota3_d, [3, NPIX])
            bcy = load(cpool, bcy_d, [2, 128])
            osel = load(cpool, osel_d, [128, 2048])
            ycol = load(cpool, ycol_d, [128, 1])
            negy = load(cpool, negy_d, [128, 1])
            ident = load(cpool, ident_d, [128, 128])
            y3h = load(cpool, y3h_d, [3, 64])

            def load2(dram, rows, cols):
                t0 = mpool.tile([128, cols], F32, tag=dram.name + "0")
                t1 = mpool.tile([128, cols], F32, tag=dram.name + "1")
                nc.sync.dma_start(t0[:], dram[0:128, :])
                nc.sync.dma_start(t1[:], dram[128:256, :])
                return t0, t1

            Wc0, Wc1 = load2(Wc_d, 256, 256)
            Ws0, Ws1 = load2(Ws_d, 256, 128)
            Wr0, Wr1 = load2(Wr_d, 256, 128)
            Wtx0, Wtx1 = load2(Wtx_d, 256, 128)
            Wty0, Wty1 = load2(Wty_d, 256, 128)
            para0, para1 = load2(para_d, 256, 1)
            bc0, bc1 = load2(bc_d, 256, 1)
            bs = load(mpool, bs_d, [128, 1])
            br = load(mpool, br_d, [128, 1])
            btx = load(mpool, btx_d, [128, 1])
            bty = load(mpool, bty_d, [128, 1])

            # ---- MLP head: p = relu(para @ Wc + bc)
            p_sb = []
            for m in range(2):
                pp = mpsum.tile([128, 1], F32, tag="pp")
                sl = slice(128 * m, 128 * (m + 1))
                nc.tensor.matmul(pp[:], Wc0[:, sl], para0[:],
                                 start=True, stop=False)
                nc.tensor.matmul(pp[:], Wc1[:, sl], para1[:],
                                 start=False, stop=True)
                pt = mpool.tile([128, 1], F32, tag=f"p{m}")
                nc.scalar.activation(pt[:], pp[:], AF.Relu,
                                     bias=(bc0 if m == 0 else bc1)[:])
                p_sb.append(pt)

            def head(W0, W1, bias, func, tag):
                ps = mpsum.tile([128, 1], F32, tag="hps")
                nc.tensor.matmul(ps[:], W0[:], p_sb[0][:],
                                 start=True, stop=False)
                nc.tensor.matmul(ps[:], W1[:], p_sb[1][:],
                                 start=False, stop=True)
                t = mpool.tile([128, 1], F32, tag=tag)
                nc.scalar.activation(t[:], ps[:], func, bias=bias[:])
                return t

            sig = head(Ws0, Ws1, bs, AF.Sigmoid, "sig")      # scale/2
            thr = head(Wr0, Wr1, br, AF.Tanh, "thr")         # angle/pi
            txv = head(Wtx0, Wtx1, btx, AF.Tanh, "txv")
            tyv = head(Wty0, Wty1, bty, AF.Tanh, "tyv")

            cosv = mpool.tile([128, 1], F32, tag="cosv")
            sinv = mpool.tile([128, 1], F32, tag="sinv")
            shalf = mpool.tile([128, 1], F32, tag="shalf")
            # cos(th) = 1 - 2 sin^2(th/2); th/2 stays within [-pi/2, pi/2]
            nc.scalar.activation(shalf[:], thr[:], AF.Sin, scale=PI / 2.0)
            nc.vector.tensor_mul(shalf[:], shalf[:], shalf[:])
            nc.vector.tensor_scalar(cosv[:], shalf[:], -2.0, 1.0,
                                    ALU.mult, ALU.add)
            nc.scalar.activation(sinv[:], thr[:], AF.Sin, scale=PI)

            # per-channel affine coefs:
            # px = ax*w + bx*h + cx ; py = ay*w + by*h + cy
            # ax = (128/63) s c ; bx = -(128/63) s s ; ay = (128/63) s s
            # by = ax ; cx = 64(ss - sc) + 32 tx + 31.5
            # cy = -64(ss + sc) + 32 ty + 31.5     (s = sigmoid, c/s = cos/sin)
            coefblk = mpool.tile([128, 8], F32, tag="coefblk")
            mc = mpool.tile([128, 1], F32, tag="mc")
            ms = mpool.tile([128, 1], F32, tag="ms")
            tmp = mpool.tile([128, 1], F32, tag="tmp")
            tmp2 = mpool.tile([128, 1], F32, tag="tmp2")
            nc.vector.tensor_mul(mc[:], sig[:], cosv[:])
            nc.vector.tensor_mul(ms[:], sig[:], sinv[:])
            K = 128.0 / 63.0
            nc.vector.tensor_scalar_mul(coefblk[:, 0:1], mc[:], K)    # ax
            nc.vector.tensor_scalar_mul(coefblk[:, 4:5], mc[:], K)    # by
            nc.vector.tensor_scalar_mul(coefblk[:, 1:2], ms[:], -K)   # bx
            nc.vector.tensor_scalar_mul(coefblk[:, 3:4], ms[:], K)    # ay
            nc.vector.tensor_sub(tmp[:], ms[:], mc[:])                # ss-sc
            nc.vector.tensor_scalar(tmp2[:], txv[:], 32.0, 31.5,
                                    ALU.mult, ALU.add)
            nc.vector.scalar_tensor_tensor(coefblk[:, 2:3], tmp[:], 64.0,
                                           tmp2[:], ALU.mult, ALU.add)  # cx
            nc.vector.tensor_add(tmp[:], ms[:], mc[:])                # ss+sc
            nc.vector.tensor_scalar(tmp2[:], tyv[:], 32.0, 31.5,
                                    ALU.mult, ALU.add)
            nc.vector.scalar_tensor_tensor(coefblk[:, 5:6], tmp[:], -64.0,
                                           tmp2[:], ALU.mult, ALU.add)  # cy
            nc.vector.tensor_scalar_mul(coefblk[:, 6:7], mc[:], 0.0)
            nc.vector.tensor_scalar_mul(coefblk[:, 7:8], mc[:], 0.0)

            # transpose coef columns -> coefTx [3, 128] (ax,bx,cx rows),
            # coefTy [3, 128] (ay,by,cy rows)
            psTx = mpsum.tile([3, 128], F32, tag="psTx")
            nc.tensor.matmul(psTx[:], coefblk[:, 0:3], ident[:],
                             start=True, stop=True)
            coefTx = mpool.tile([3, 128], F32, tag="coefTx")
            nc.vector.tensor_copy(coefTx[:], psTx[:])
            psTy = mpsum.tile([3, 128], F32, tag="psTy")
            nc.tensor.matmul(psTy[:], coefblk[:, 3:6], ident[:],
                             start=True, stop=True)
            coefTy = mpool.tile([3, 128], F32, tag="coefTy")
            nc.vector.tensor_copy(coefTy[:], psTy[:])

            mlp_psum_scope.__exit__(None, None, None)

            main_psum_scope = [
                tc.tile_pool(name="psumA", bufs=2, space="PSUM"),
                tc.tile_pool(name="psumB", bufs=2, space="PSUM"),
                tc.tile_pool(name="psumG", bufs=2, space="PSUM"),
                tc.tile_pool(name="psumO", bufs=2, space="PSUM"),
            ]
            psA_pool, psB_pool, psG_pool, psO_pool = [
                s.__enter__() for s in main_psum_scope]


            # ---- z-blend (and build block-diagonal lhsT tiles)
            bd_all = bpool.tile([128, NPAIR * 128], F32, tag="bd")
            nc.gpsimd.memset(bd_all[:], 0.0)
            bdv = bd_all[:].rearrange("p (r c) -> p r c", c=128)
            BL = 1024  # 16 pairs per blend chunk
            with tc.tile_pool(name="blendp", bufs=2) as blp:
                for bi in range(NPIX // BL):
                    sl = slice(bi * BL, (bi + 1) * BL)
                    rsl = slice(bi * BL // 64, (bi + 1) * BL // 64)
                    cur = blp.tile([128, BL], F32, tag="cur")
                    nc.sync.dma_start(cur[:], fmt_d[:, sl])
                    oth = blp.tile([128, BL], F32, tag="oth")
                    nc.sync.dma_start(oth[:], fmo_d[:, sl])
                    zcc = blp.tile([128, BL], F32, tag="zcc")
                    nc.sync.dma_start(zcc[:], zcc_d[:, sl])
                    zco = blp.tile([128, BL], F32, tag="zco")
                    nc.sync.dma_start(zco[:], zco_d[:, sl])
                    t1 = blp.tile([128, BL], F32, tag="bt1")
                    nc.vector.tensor_mul(t1[:], oth[:], zco[:])
                    t2 = blp.tile([128, BL], F32, tag="bt2")
                    nc.vector.tensor_mul(t2[:], cur[:], zcc[:])
                    nc.vector.tensor_add(t1[:], t1[:], t2[:])
                    blv = t1[:].rearrange("p (r x) -> p r x", x=64)
                    nc.vector.tensor_copy(bdv[0:64, rsl, 0:64], blv[0:64])
                    nc.vector.tensor_copy(bdv[64:128, rsl, 64:128], blv[64:128])

            # ---- main loop: batches of 16 pairs; per-pair lhsT tiles
            # [3, 128] hold (a, b, c - y) so psA/psB are already py-y / px-x
            out_sb = bpool.tile([128, NPIX], F32, tag="out")
            nc.gpsimd.memset(out_sb[:], 0.0)
            NB = 16
            for bat in range(NPAIR // NB):
                g = bat // 2                      # osel row group
                lhsp = bpool.tile([3, NB * 2 * 128], F32, tag="lhsp",
                                  bufs=2)
                for rl in range(NB):
                    r = bat * NB + rl
                    for coord, cT in ((0, coefTy), (1, coefTx)):
                        col = (2 * rl + coord) * 128
                        for hf in range(2):
                            nc.vector.tensor_scalar(
                                lhsp[:, col + 64 * hf: col + 64 * hf + 64],
                                y3h[:], cT[:, 2 * r + hf: 2 * r + hf + 1],
                                None, ALU.add)
                for ci in range(NCHUNK):
                    sl = slice(ci * CHUNK, (ci + 1) * CHUNK)
                    psO = psO_pool.tile([128, CHUNK], F32, tag="psO")
                    for rl in range(NB):
                        r = bat * NB + rl
                        psA = psA_pool.tile([128, CHUNK], F32, tag="psA")
                        nc.tensor.matmul(psA[:],
                                         lhsp[:, 2 * rl * 128:
                                              2 * rl * 128 + 128],
                                         iota3[:, sl], start=True, stop=True)
                        psB = psB_pool.tile([128, CHUNK], F32, tag="psB")
                        nc.tensor.matmul(psB[:],
                                         lhsp[:, (2 * rl + 1) * 128:
                                              (2 * rl + 1) * 128 + 128],
                                         iota3[:, sl], start=True, stop=True)
                        # S = min(|py - y|, 1) - 1 = -tri_y
                        Sa = wpool.tile([128, CHUNK], F32, tag="Sa")
                        nc.scalar.activation(Sa[:], psA[:], AF.Abs)
                        S = wpool.tile([128, CHUNK], F32, tag="S")
                        nc.gpsimd.tensor_scalar(S[:], Sa[:], 1.0, 1.0,
                                                ALU.min, ALU.subtract)
                        # W1 = min(|px - x|, 1) - 1 = -tri_x
                        Wa = wpool.tile([128, CHUNK], F32, tag="Wa")
                        nc.scalar.activation(Wa[:], psB[:], AF.Abs)
                        W1 = wpool.tile([128, CHUNK], F32, tag="W1")
                        nc.vector.tensor_scalar(W1[:], Wa[:], 1.0, 1.0,
                                                ALU.min, ALU.subtract)
                        psG = psG_pool.tile([128, CHUNK], F32, tag="psG")
                        nc.tensor.matmul(
                            psG[:], bd_all[:, r * 128:(r + 1) * 128], S[:],
                            start=True, stop=True)
                        P = wpool.tile([128, CHUNK], F32, tag="P")
                        nc.vector.tensor_mul(P[:], psG[:], W1[:])
                        v = r % 32
                        nc.tensor.matmul(psO[64 * g:64 * g + 64, :],
                                         osel[:, 64 * v:64 * v + 64], P[:],
                                         start=(rl == 0), stop=(rl == NB - 1))
                    # out += psO  (both tris negated, signs cancel)
                    nc.vector.tensor_add(out_sb[64 * g:64 * g + 64, sl],
                                         out_sb[64 * g:64 * g + 64, sl],
                                         psO[64 * g:64 * g + 64, :])
            nc.sync.dma_start(out_d[:, :], out_sb[:])
            for s in reversed(main_psum_scope):
                s.__exit__(None, None, None)

    nc.compile()
    return nc


_NC_CACHE = None


def _get_nc():
    global _NC_CACHE
    if _NC_CACHE is None:
        _NC_CACHE = build_nc()
    return _NC_CACHE


def kernel(**inputs):
    nc = _get_nc()
    in_maps = shard_inputs(**inputs)
    res = run_bass_kernel_spmd(nc, in_maps, core_ids=list(range(8)))
    out = np.zeros((B, C, H, W), np.float32)
    for core in range(8):
        b_i, half = core // 2, core % 2
        out[b_i, 128 * half:128 * (half + 1)] = (
            res.results[core]["out"].reshape(128, H, W))
    return out
